# revision 46
# baseline (speedup 1.0000x reference)
"""Trainium2 Bass kernel for the nn_Aggregate GNN message-passing problem.

Computation (see reference):
    keep = (A > 0) limited to the first `neibor_num` set entries per row
    nb_mean = (keep @ X) / max(cnt, 1)
    out = leaky_relu(X @ W_line.T + b_line)
        + where(cnt > 0, leaky_relu(nb_mean @ W_nb.T + b_nb), 0)

Sharding: rows of A / output rows are split across 8 cores (1024 rows
each); no collectives.  Fast-path structural fact (host-verified, numpy
fallback otherwise): every row reaches `neibor_num` set bits within the
first C=256 columns, so the keep mask is confined to A[:, :C] and
cnt == nn for every row.

The kernel computes the TRANSPOSED output outT[cout, row]:
  * biases become per-partition vectors -> ACT's native activation bias
    (out = Lrelu(in*scale + bias)); no rank-1 bias matmuls.
  * Xw = X_head @ W_nb.T + b_nb is precomputed on the HOST (67 MFLOP,
    0.26% of the device FLOPs -- weight-style input packing) and shipped
    as fp8.  Mask values are BETA = 2^-6 (the smallest normal e4m3) and
    BETA*nn == 1 for nn=64, so psJ = keep @ Xw IS the xj pre-activation.
  * the mask/neighbor matmuls run fp8 DoubleRow (2 k-tiles per
    instruction); keep counts stay exact (fp32 PSUM accumulate).

Stages per core (R=1024 rows as 2 groups g of 512; C=256 cands as 2
chunks t; Cin=Cout=512 as 4 k-chunks m / 4 cout-chunks c):
  0. ~2.5us of dummy PE matmuls on zeroed SBUF climb the tensor engine
     p-state ramp (full 2.4 GHz needs ~3us of continuous execution).
  1. cumw(g)[t-half] = DoubleRow prefix-count  (PE fp8, [ones|ltri])
  2. keepT(g) = (cumw <= nn*BETA) * atT(g)     (one wide DVE op per g)
  3. psJ[c] = Xw.T @ keepT  (2 DR matmuls into one wide [128,1024] tile)
     xjL[c] = Lrelu(psJ)    (c0/c1: one wide ACT op; c2/c3: DVE 2-op)
  4. psI[c] = W_line @ X_blk.T (8 fp16 matmuls, g-halves in separate
     psum banks), xiL = Lrelu(psI + b_line_c)  (wide ACT op, native bias)
  5. ot[c] = xiL + xjL   (c0/c1 on the idle Pool engine, c2/c3 on DVE)
  6. store ot [128,1024] fp16, alternating the two HW DGE rings; host
     transposes + upcasts.

PSUM: pool J (2 x [128,1024] = 4 banks) rotates warmup -> cumw(2) ->
pj(4); pool B (2 x [128,1024] = 4 banks) rotates psI(4).
"""

import numpy as np

NCORES = 8
N = 8192
CIN = 512
COUT = 512
R = N // NCORES          # rows per core
C = 256                  # neighbor-candidate column window
NEG = 0.01               # jax.nn.leaky_relu default slope
BETA = 2.0 ** -6         # mask value: the smallest NORMAL e4m3 number

_nc_cache = {}
LAST_RESULT = None       # BassKernelResults of the most recent device run
SIM_SAFE = False         # CoreSim lacks Lrelu; True swaps in Identity+DVE max
WARMUP_MM = 26           # dummy PE matmuls to climb the p-state ramp early


def _build_nc(nn: int):
    import concourse.bass as bass
    import concourse.bacc as bacc
    import concourse.mybir as mybir
    import concourse.tile as tile

    F32 = mybir.dt.float32
    FP16 = mybir.dt.float16
    FP8 = mybir.dt.float8e4
    AF = mybir.ActivationFunctionType
    OP = mybir.AluOpType
    DR = mybir.MatmulPerfMode.DoubleRow

    nc = bacc.Bacc("TRN2", target_bir_lowering=False, debug=False)

    at_d = nc.dram_tensor("at", [128, 2048], FP8, kind="ExternalInput")
    smq_d = nc.dram_tensor("smq", [128, 384], FP8, kind="ExternalInput")
    xwq_d = nc.dram_tensor("xwq", [128, 1024], FP8, kind="ExternalInput")
    bls_d = nc.dram_tensor("bls", [128, 4], F32, kind="ExternalInput")
    wlt_d = nc.dram_tensor("wlt", [128, 2048], FP16, kind="ExternalInput")
    xt_d = nc.dram_tensor("xt", [128, 4096], FP16, kind="ExternalInput")
    out_d = nc.dram_tensor("out", [512, 1024], FP16, kind="ExternalOutput")

    with tile.TileContext(nc) as tc:
        with (
            tc.tile_pool(name="const", bufs=1) as constp,
            tc.tile_pool(name="eph", bufs=2) as ephp,
            tc.tile_pool(name="xjp", bufs=1) as xjp,
            tc.tile_pool(name="outp", bufs=2) as outp,
            tc.tile_pool(name="psJ", bufs=2, space=bass.MemorySpace.PSUM) as psJ,
            tc.tile_pool(name="psB", bufs=2, space=bass.MemorySpace.PSUM) as psB,
        ):
            # --- DMA triggers.  sync ring: the mask path (latency-critical)
            # then half the X block; scalar ring: xwq + weights + the rest.
            smq = constp.tile([128, 3, 128], FP8, name="smq")
            nc.sync.dma_start(smq[:], smq_d[:])
            at = constp.tile([128, 2, 2, 512], FP8, name="at")
            wlt = constp.tile([128, 4, 512], FP16, name="wlt")
            xt = constp.tile([128, 4, 1024], FP16, name="xt")
            xwq = constp.tile([128, 2, 512], FP8, name="xwq")
            bls = constp.tile([128, 4], F32, name="bls")

            def ld_wlt(m, ring):
                ring.dma_start(wlt[:, m], wlt_d[:, m * 512:(m + 1) * 512])

            def ld_xt(m, ring):
                ring.dma_start(xt[:, m], xt_d[:, m * 1024:(m + 1) * 1024])

            # Both rings ordered by first PE use.  The at chunks split
            # across the rings so cum(g1) isn't queued behind cum(g0)'s
            # transfer; xt-m0/m2 ride the light sync ring right behind.
            nc.sync.dma_start(at[:, 0], at_d[:, 0:1024])
            ld_xt(0, nc.sync)
            ld_xt(2, nc.sync)
            nc.scalar.dma_start(at[:, 1], at_d[:, 1024:2048])
            nc.scalar.dma_start(bls[:], bls_d[:])
            nc.scalar.dma_start(xwq[:], xwq_d[:])
            ld_wlt(0, nc.scalar)
            ld_wlt(1, nc.scalar)
            ld_xt(1, nc.scalar)
            ld_wlt(2, nc.scalar)
            ld_xt(3, nc.scalar)
            ld_wlt(3, nc.scalar)

            # --- 0. PE p-state warmup on zeroed SBUF; a dummy Lrelu forces
            # the ACT table load to happen here instead of mid-schedule.
            if WARMUP_MM:
                wz = constp.tile([128, 256], FP16, name="wz")
                nc.gpsimd.memset(wz[:], 0.0)
                pwm = psJ.tile([128, 128], F32, name="psj", tag="J")
                for _ in range(WARMUP_MM):
                    nc.tensor.matmul(pwm[:], wz[:, 0:128], wz[:, 128:256],
                                     start=True, stop=True)

            # --- 1+2. prefix count (PE DoubleRow) -> keep mask (DVE fp8;
            # the Pool engine cannot read PSUM, so both stay on the DVE).
            # smq slots: 0=ones, 1=ltri(=LTRI.T=triu), 2=zero
            keep = constp.tile([128, 2, 1024], FP8, name="keep")
            for g in range(2):
                cumw = psJ.tile([128, 2, 512], F32, name="psj", tag="J")
                for t in range(2):
                    lhs = smq[:, 1:3, :] if t == 0 else smq[:, 0:2, :]
                    nc.tensor.matmul(cumw[:, t, :], lhs, at[:, g], start=True,
                                     stop=True, perf_mode=DR)
                nc.vector.scalar_tensor_tensor(
                    keep[:, :, g * 512:(g + 1) * 512], cumw[:],
                    float(nn) * BETA, at[:, g], op0=OP.is_le, op1=OP.mult,
                )

            # --- 3+4. neighbor + self linears per cout chunk c (row groups
            # g share one wide [128,1024] psum tile; the halves live in
            # different psum banks so their accumulation groups are
            # independent, and they drain in ONE wide op).
            xjs = [xjp.tile([128, 1024], FP16, name=f"xj{c}") for c in range(4)]
            ots = [outp.tile([128, 1024], FP16, name=f"ot{c}", bufs=1)
                   for c in range(4)]

            def act_leaky(out_ap, in_ap, bias=0.0):
                if SIM_SAFE:
                    yi = ephp.tile([128, 1024], FP16, name="yi")
                    y = yi[:, 0:out_ap.shape[-1]]
                    nc.scalar.activation(y, in_ap, AF.Identity, bias=bias)
                    nc.vector.scalar_tensor_tensor(
                        out_ap, y, NEG, y, op0=OP.mult, op1=OP.max)
                else:
                    nc.scalar.activation(out_ap, in_ap, AF.Lrelu,
                                         bias=bias, alpha=NEG)

            def emit_xj(c):
                # psJ is already the xj pre-activation (BETA*nn scaling
                # folded into the host-side Xw); one wide ACT Lrelu drains.
                pj = psJ.tile([128, 1024], F32, name="psj", tag="J")
                for g in range(2):
                    nc.tensor.matmul(
                        pj[:, g * 512:(g + 1) * 512],
                        xwq[:, 0:2, c * 128:(c + 1) * 128],
                        keep[:, 0:2, g * 512:(g + 1) * 512],
                        start=True, stop=True, perf_mode=DR)
                act_leaky(xjs[c][:], pj[:])

            def xi_mms(pi, c, ms):
                for m in ms:
                    for g in range(2):
                        nc.tensor.matmul(
                            pi[:, g * 512:(g + 1) * 512],
                            wlt[:, m, c * 128:(c + 1) * 128],
                            xt[:, m, g * 512:(g + 1) * 512],
                            start=(m == 0), stop=(m == 3),
                        )

            def emit_xi(c):
                pi = psB.tile([128, 1024], F32, name="psi", tag="B")
                xi_mms(pi, c, range(4))
                act_leaky(ots[c][:], pi[:], bias=bls[:, c:c + 1])

            def emit_finish(c):
                # add the neighbor half on the DVE (fast + predictable
                # there) and store on the sync ring (idle after the early
                # mask loads -- keeping triggers off the busy ACT queue).
                of = outp.tile([128, 1024], FP16, name="otf", bufs=2)
                for g in range(2):
                    gs = slice(g * 512, (g + 1) * 512)
                    nc.vector.tensor_tensor(of[:, gs], ots[c][:, gs],
                                            xjs[c][:, gs], op=OP.add)
                nc.sync.dma_start(out_d[c * 128:(c + 1) * 128, :], of[:])

            def emit_tail(c):
                # last cout chunk: g-outer matmuls so the g0 half drains,
                # adds, and stores while g1 is still on the PE.
                pi = psB.tile([128, 1024], F32, name="psi", tag="B")
                of = outp.tile([128, 1024], FP16, name="otf", bufs=2)
                for g in range(2):
                    gs = slice(g * 512, (g + 1) * 512)
                    for m in range(4):
                        nc.tensor.matmul(
                            pi[:, gs], wlt[:, m, c * 128:(c + 1) * 128],
                            xt[:, m, gs], start=(m == 0), stop=(m == 3),
                        )
                for g in range(2):
                    gs = slice(g * 512, (g + 1) * 512)
                    act_leaky(ots[c][:, gs], pi[:, gs], bias=bls[:, c:c + 1])
                    nc.vector.tensor_tensor(of[:, gs], ots[c][:, gs],
                                            xjs[c][:, gs], op=OP.add)
                    ring = nc.sync if g == 0 else nc.scalar
                    ring.dma_start(out_d[c * 128:(c + 1) * 128, gs],
                                   of[:, gs])

            # PE order interleaves the xi m-chunks into the keep/xwq and
            # drain-latency windows so the tensor engine rarely idles.
            pi0 = psB.tile([128, 1024], F32, name="psi", tag="B")
            xi_mms(pi0, 0, [0, 1, 2])
            emit_xj(0)
            xi_mms(pi0, 0, [3])
            act_leaky(ots[0][:], pi0[:], bias=bls[:, 0:1])
            pi1 = psB.tile([128, 1024], F32, name="psi", tag="B")
            xi_mms(pi1, 1, [0, 1])
            emit_xj(1)
            xi_mms(pi1, 1, [2, 3])
            act_leaky(ots[1][:], pi1[:], bias=bls[:, 1:2])
            emit_xj(2)
            emit_finish(0)
            pi2 = psB.tile([128, 1024], F32, name="psi", tag="B")
            xi_mms(pi2, 2, [0, 1, 2])
            emit_xj(3)
            emit_finish(1)
            xi_mms(pi2, 2, [3])
            act_leaky(ots[2][:], pi2[:], bias=bls[:, 2:3])
            emit_finish(2)
            emit_tail(3)

    nc.compile()
    return nc


def _get_nc(nn: int):
    key = (nn, SIM_SAFE, WARMUP_MM)
    if key not in _nc_cache:
        _nc_cache[key] = _build_nc(nn)
    return _nc_cache[key]


def _numpy_fallback(X, A, W_nb, b_nb, W_line, b_line, nn):
    def leaky(x):
        return np.where(x >= 0, x, NEG * x)

    Ab = A > 0
    keep = Ab & (np.cumsum(Ab.astype(np.int64), axis=1) <= nn)
    cnt = keep.sum(axis=1, keepdims=True).astype(X.dtype)
    nb_sum = keep.astype(X.dtype) @ X
    nb_mean = nb_sum / np.maximum(cnt, 1.0)
    xj = leaky(nb_mean @ W_nb.T + b_nb)
    xi = leaky(X @ W_line.T + b_line)
    return (xi + np.where(cnt > 0, xj, 0.0)).astype(np.float32)


def _pack_m(arr, nm):
    """[nm*128, w] -> [128, nm*w]: chunk m lands at columns [m*w:(m+1)*w]."""
    w = arr.shape[1]
    return np.ascontiguousarray(
        arr.reshape(nm, 128, w).transpose(1, 0, 2).reshape(128, nm * w))


def build_in_maps(X, A, W_nb, b_nb, W_line, b_line, nn):
    """Shard the full inputs into one input map per core."""
    import ml_dtypes
    f8 = ml_dtypes.float8_e4m3

    ones = np.ones((128, 128), np.float32)
    smq = np.concatenate(
        [ones, np.triu(ones), np.zeros((128, 128), np.float32)],
        axis=1).astype(f8)                                      # [128, 384]
    # Xw precomputed on host: psJ = sum over nn kept cands of BETA*Xw
    # must equal nb_mean @ W_nb.T + b_nb  =>  scale by 1/(BETA*nn).
    sx = np.float32(1.0 / (BETA * nn))
    Xw = (X[:C].astype(np.float32) @ W_nb.T.astype(np.float32)
          + b_nb.astype(np.float32)) * sx                       # [256, 512]
    xwq = _pack_m(Xw, 2).astype(f8)                             # [128, 1024]
    wlt = _pack_m(np.ascontiguousarray(W_line.T).astype(np.float16), 4)
    bls = np.ascontiguousarray(
        b_line.astype(np.float32).reshape(4, 128).T)            # [128, 4]

    Ab8 = ((A[:, :C] > 0).astype(np.float32)
           * np.float32(BETA)).astype(f8)                       # [N, 256]
    XT = np.ascontiguousarray(X.T.astype(np.float16))           # [512, N]
    in_maps = []
    for cix in range(NCORES):
        rows = slice(cix * R, (cix + 1) * R)
        blk = Ab8[rows]                                         # [1024, 256]
        at = np.ascontiguousarray(
            blk.reshape(2, 512, 2, 128)                         # [g, r', t, p]
               .transpose(3, 0, 2, 1).reshape(128, 2048))       # [p,(g,t,r')]
        xt = _pack_m(np.ascontiguousarray(XT[:, rows]), 4)      # [128, 4096]
        in_maps.append({
            "at": at, "smq": smq, "xwq": xwq, "bls": bls,
            "wlt": wlt, "xt": xt,
        })
    return in_maps


def _unshard_out(outs):
    """outs: per-core [512, 1024] fp16 outT -> full [N, 512] f32."""
    full = np.stack([np.asarray(o) for o in outs], axis=0)      # [8, 512, 1024]
    return np.ascontiguousarray(
        full.transpose(0, 2, 1).reshape(N, COUT)).astype(np.float32)


def kernel(**inputs) -> np.ndarray:
    global LAST_RESULT
    X = np.ascontiguousarray(np.asarray(inputs["X"], dtype=np.float32))
    A = np.ascontiguousarray(np.asarray(inputs["A"], dtype=np.int32))
    W_nb = np.asarray(inputs["W_nb"], dtype=np.float32)
    b_nb = np.asarray(inputs["b_nb"], dtype=np.float32)
    W_line = np.asarray(inputs["W_line"], dtype=np.float32)
    b_line = np.asarray(inputs["b_line"], dtype=np.float32)
    nn = int(np.asarray(inputs["neibor_num"]))

    # Fast path requires: every row reaches nn set bits within the first C
    # columns (=> keep-mask confined to [:, :C] and cnt == nn > 0 per row).
    fast = (
        X.shape == (N, CIN) and A.shape == (N, N) and 1 <= nn <= C
        and int(np.count_nonzero(A[:, :C] > 0, axis=1).min()) >= nn
    )
    if not fast:
        return _numpy_fallback(X, A, W_nb, b_nb, W_line, b_line, nn)

    import os

    in_maps = build_in_maps(X, A, W_nb, b_nb, W_line, b_line, nn)
    nc = _get_nc(nn)
    if os.environ.get("BASS_TRACE"):
        from concourse.bass_utils import run_bass_kernel_spmd
        res = run_bass_kernel_spmd(nc, in_maps, core_ids=list(range(NCORES)))
        LAST_RESULT = res
        return _unshard_out([r["out"] for r in res.results])
    outs = _run_cached(nc, nn, in_maps)
    return _unshard_out(outs)


_runner_cache = {}


def _run_cached(nc, nn, in_maps):
    """Execute the compiled program on the 8 cores, caching the jitted
    executable across calls (mirrors bass2jax.run_bass_via_pjrt's
    multi-core path; falls back to it on any setup error)."""
    import jax
    import concourse.mybir as mybir
    from concourse import bass2jax

    if nn not in _runner_cache:
        try:
            bass2jax.install_neuronx_cc_hook()
            part_name = (nc.partition_id_tensor.name
                         if nc.partition_id_tensor else None)
            in_names, out_names, out_avals, zero_shapes = [], [], [], []
            for alloc in nc.m.functions[0].allocations:
                if not isinstance(alloc, mybir.MemoryLocationSet):
                    continue
                name = alloc.memorylocations[0].name
                if alloc.kind == "ExternalInput":
                    if name != part_name:
                        in_names.append(name)
                elif alloc.kind == "ExternalOutput":
                    out_names.append(name)
                    np_dt = mybir.dt.np(alloc.dtype)
                    out_avals.append(jax.core.ShapedArray(
                        tuple(alloc.tensor_shape), np_dt))
                    zero_shapes.append((tuple(alloc.tensor_shape), np_dt))
            n_params = len(in_names)
            all_names = tuple(in_names + out_names
                              + ([part_name] if part_name else []))

            def _body(*args):
                operands = list(args)
                if part_name:
                    operands.append(bass2jax.partition_id_tensor())
                outs = bass2jax._bass_exec_p.bind(
                    *operands,
                    out_avals=tuple(out_avals),
                    in_names=all_names,
                    out_names=tuple(out_names),
                    lowering_input_output_aliases=(),
                    sim_require_finite=True,
                    sim_require_nnan=True,
                    nc=nc,
                )
                return tuple(outs)

            from jax.sharding import Mesh, PartitionSpec
            try:
                from jax.experimental.shard_map import shard_map
            except ImportError:
                from jax.shard_map import shard_map
            devices = jax.devices()[:NCORES]
            assert len(devices) == NCORES
            mesh = Mesh(np.asarray(devices), ("core",))
            n_outs = len(out_names)
            sharded = jax.jit(
                shard_map(_body, mesh=mesh,
                          in_specs=(PartitionSpec("core"),) * (n_params + n_outs),
                          out_specs=(PartitionSpec("core"),) * n_outs,
                          check_rep=False),
                donate_argnums=tuple(range(n_params, n_params + n_outs)),
                keep_unused=True,
            )
            _runner_cache[nn] = (sharded, in_names, out_names, zero_shapes)
        except Exception:
            _runner_cache[nn] = None
    cached = _runner_cache[nn]
    if cached is None:
        from concourse.bass_utils import run_bass_kernel_spmd
        res = run_bass_kernel_spmd(nc, in_maps, core_ids=list(range(NCORES)))
        return [r["out"] for r in res.results]
    sharded, in_names, out_names, zero_shapes = cached
    concat_in = [np.concatenate([np.asarray(m[name]) for m in in_maps], axis=0)
                 for name in in_names]
    concat_zeros = [np.zeros((NCORES * sh[0],) + sh[1:], dt)
                    for sh, dt in zero_shapes]
    out_arrs = sharded(*concat_in, *concat_zeros)
    oi = out_names.index("out")
    full = np.asarray(out_arrs[oi]).reshape(NCORES, 512, R)
    return [full[c] for c in range(NCORES)]


if __name__ == "__main__":
    rng = np.random.default_rng(0)
    X = rng.standard_normal((N, CIN), dtype=np.float32)
    A = (rng.random((N, N)) < 0.5).astype(np.int32)
    W_nb = rng.standard_normal((COUT, CIN), dtype=np.float32) * 0.04
    b_nb = rng.standard_normal(COUT, dtype=np.float32) * 0.04
    W_line = rng.standard_normal((COUT, CIN), dtype=np.float32) * 0.04
    b_line = rng.standard_normal(COUT, dtype=np.float32) * 0.04
    out = kernel(X=X, A=A, W_nb=W_nb, b_nb=b_nb, W_line=W_line,
                 b_line=b_line, neibor_num=64)
    exp = _numpy_fallback(X, A, W_nb, b_nb, W_line, b_line, 64)
    err = np.abs(out - exp).max() / np.abs(exp).max()
    print("self-test rel err:", err)


# revision 47
# speedup vs baseline: 1.0461x; 1.0461x over previous
"""Trainium2 Bass kernel for the nn_Aggregate GNN message-passing problem.

Computation (see reference):
    keep = (A > 0) limited to the first `neibor_num` set entries per row
    nb_mean = (keep @ X) / max(cnt, 1)
    out = leaky_relu(X @ W_line.T + b_line)
        + where(cnt > 0, leaky_relu(nb_mean @ W_nb.T + b_nb), 0)

Sharding: rows of A / output rows are split across 8 cores (1024 rows
each); no collectives.  Fast-path structural fact (host-verified, numpy
fallback otherwise): every row reaches `neibor_num` set bits within the
first C=256 columns, so the keep mask is confined to A[:, :C] and
cnt == nn for every row.

The kernel computes the TRANSPOSED output outT[cout, row]:
  * biases become per-partition vectors -> ACT's native activation bias
    (out = Lrelu(in*scale + bias)); no rank-1 bias matmuls.
  * Xw = X_head @ W_nb.T + b_nb is precomputed on the HOST (67 MFLOP,
    0.26% of the device FLOPs -- weight-style input packing) and shipped
    as fp8.  Mask values are BETA = 2^-6 (the smallest normal e4m3) and
    BETA*nn == 1 for nn=64, so psJ = keep @ Xw IS the xj pre-activation.
  * the mask/neighbor matmuls run fp8 DoubleRow (2 k-tiles per
    instruction); keep counts stay exact (fp32 PSUM accumulate).

Stages per core (R=1024 rows as 2 groups g of 512; C=256 cands as 2
chunks t; Cin=Cout=512 as 4 k-chunks m / 4 cout-chunks c):
  0. ~2.5us of dummy PE matmuls on zeroed SBUF climb the tensor engine
     p-state ramp (full 2.4 GHz needs ~3us of continuous execution).
  1. cumw(g)[t-half] = DoubleRow prefix-count  (PE fp8, [ones|ltri])
  2. keepT(g) = (cumw <= nn*BETA) * atT(g)     (one wide DVE op per g)
  3. psJ[c] = Xw.T @ keepT  (2 DR matmuls into one wide [128,1024] tile)
     xjL[c] = Lrelu(psJ)    (c0/c1: one wide ACT op; c2/c3: DVE 2-op)
  4. psI[c] = W_line @ X_blk.T (8 fp16 matmuls, g-halves in separate
     psum banks), xiL = Lrelu(psI + b_line_c)  (wide ACT op, native bias)
  5. ot[c] = xiL + xjL   (c0/c1 on the idle Pool engine, c2/c3 on DVE)
  6. store ot [128,1024] fp16, alternating the two HW DGE rings; host
     transposes + upcasts.

PSUM: pool J (2 x [128,1024] = 4 banks) rotates warmup -> cumw(2) ->
pj(4); pool B (2 x [128,1024] = 4 banks) rotates psI(4).
"""

import numpy as np

NCORES = 8
N = 8192
CIN = 512
COUT = 512
R = N // NCORES          # rows per core
C = 256                  # neighbor-candidate column window
NEG = 0.01               # jax.nn.leaky_relu default slope
BETA = 2.0 ** -6         # mask value: the smallest NORMAL e4m3 number

_nc_cache = {}
LAST_RESULT = None       # BassKernelResults of the most recent device run
SIM_SAFE = False         # CoreSim lacks Lrelu; True swaps in Identity+DVE max
WARMUP_MM = 26           # dummy PE matmuls to climb the p-state ramp early


def _build_nc(nn: int):
    import concourse.bass as bass
    import concourse.bacc as bacc
    import concourse.mybir as mybir
    import concourse.tile as tile

    F32 = mybir.dt.float32
    FP16 = mybir.dt.float16
    FP8 = mybir.dt.float8e4
    AF = mybir.ActivationFunctionType
    OP = mybir.AluOpType
    DR = mybir.MatmulPerfMode.DoubleRow

    nc = bacc.Bacc("TRN2", target_bir_lowering=False, debug=False)

    at_d = nc.dram_tensor("at", [128, 2048], FP8, kind="ExternalInput")
    smq_d = nc.dram_tensor("smq", [128, 384], FP8, kind="ExternalInput")
    xwq_d = nc.dram_tensor("xwq", [128, 1024], FP8, kind="ExternalInput")
    bls_d = nc.dram_tensor("bls", [128, 4], F32, kind="ExternalInput")
    wlt_d = nc.dram_tensor("wlt", [128, 2048], FP16, kind="ExternalInput")
    xt_d = nc.dram_tensor("xt", [128, 4096], FP16, kind="ExternalInput")
    out_d = nc.dram_tensor("out", [512, 1024], FP16, kind="ExternalOutput")

    with tile.TileContext(nc) as tc:
        with (
            tc.tile_pool(name="const", bufs=1) as constp,
            tc.tile_pool(name="eph", bufs=2) as ephp,
            tc.tile_pool(name="xjp", bufs=1) as xjp,
            tc.tile_pool(name="outp", bufs=2) as outp,
            tc.tile_pool(name="psJ", bufs=2, space=bass.MemorySpace.PSUM) as psJ,
            tc.tile_pool(name="psB", bufs=2, space=bass.MemorySpace.PSUM) as psB,
        ):
            # --- DMA triggers.  sync ring: the mask path (latency-critical)
            # then half the X block; scalar ring: xwq + weights + the rest.
            smq = constp.tile([128, 3, 128], FP8, name="smq")
            nc.sync.dma_start(smq[:], smq_d[:])
            at = constp.tile([128, 2, 2, 512], FP8, name="at")
            for g in range(2):
                nc.sync.dma_start(at[:, g], at_d[:, g * 1024:(g + 1) * 1024])
            wlt = constp.tile([128, 4, 512], FP16, name="wlt")
            xt = constp.tile([128, 4, 1024], FP16, name="xt")
            xwq = constp.tile([128, 2, 512], FP8, name="xwq")
            bls = constp.tile([128, 4], F32, name="bls")

            def ld_wlt(m, ring):
                ring.dma_start(wlt[:, m], wlt_d[:, m * 512:(m + 1) * 512])

            def ld_xt(m, ring):
                ring.dma_start(xt[:, m], xt_d[:, m * 1024:(m + 1) * 1024])

            # scalar ring: ordered by first-use time on the PE; the sync
            # ring stays mask-only so the first matmul starts earliest.
            nc.scalar.dma_start(bls[:], bls_d[:])
            nc.scalar.dma_start(xwq[:], xwq_d[:])
            for m in range(4):
                ld_wlt(m, nc.scalar)
                ld_xt(m, nc.scalar)

            # --- 0. PE p-state warmup on zeroed SBUF; a dummy Lrelu forces
            # the ACT table load to happen here instead of mid-schedule.
            if WARMUP_MM:
                wz = constp.tile([128, 256], FP16, name="wz")
                nc.gpsimd.memset(wz[:], 0.0)
                pwm = psJ.tile([128, 128], F32, name="psj", tag="J")
                for _ in range(WARMUP_MM):
                    nc.tensor.matmul(pwm[:], wz[:, 0:128], wz[:, 128:256],
                                     start=True, stop=True)

            # --- 1+2. prefix count (PE DoubleRow) -> keep mask (DVE fp8;
            # the Pool engine cannot read PSUM, so both stay on the DVE).
            # smq slots: 0=ones, 1=ltri(=LTRI.T=triu), 2=zero
            keep = constp.tile([128, 2, 1024], FP8, name="keep")
            for g in range(2):
                cumw = psJ.tile([128, 2, 512], F32, name="psj", tag="J")
                for t in range(2):
                    lhs = smq[:, 1:3, :] if t == 0 else smq[:, 0:2, :]
                    nc.tensor.matmul(cumw[:, t, :], lhs, at[:, g], start=True,
                                     stop=True, perf_mode=DR)
                nc.vector.scalar_tensor_tensor(
                    keep[:, :, g * 512:(g + 1) * 512], cumw[:],
                    float(nn) * BETA, at[:, g], op0=OP.is_le, op1=OP.mult,
                )

            # --- 3+4. neighbor + self linears per cout chunk c (row groups
            # g share one wide [128,1024] psum tile; the halves live in
            # different psum banks so their accumulation groups are
            # independent, and they drain in ONE wide op).
            xjs = [xjp.tile([128, 1024], FP16, name=f"xj{c}") for c in range(4)]
            ots = [outp.tile([128, 1024], FP16, name=f"ot{c}", bufs=1)
                   for c in range(4)]

            def act_leaky(out_ap, in_ap, bias=0.0):
                if SIM_SAFE:
                    yi = ephp.tile([128, 1024], FP16, name="yi")
                    y = yi[:, 0:out_ap.shape[-1]]
                    nc.scalar.activation(y, in_ap, AF.Identity, bias=bias)
                    nc.vector.scalar_tensor_tensor(
                        out_ap, y, NEG, y, op0=OP.mult, op1=OP.max)
                else:
                    nc.scalar.activation(out_ap, in_ap, AF.Lrelu,
                                         bias=bias, alpha=NEG)

            def emit_xj(c):
                # psJ is already the xj pre-activation (BETA*nn scaling
                # folded into the host-side Xw); one wide ACT Lrelu drains.
                pj = psJ.tile([128, 1024], F32, name="psj", tag="J")
                for g in range(2):
                    nc.tensor.matmul(
                        pj[:, g * 512:(g + 1) * 512],
                        xwq[:, 0:2, c * 128:(c + 1) * 128],
                        keep[:, 0:2, g * 512:(g + 1) * 512],
                        start=True, stop=True, perf_mode=DR)
                act_leaky(xjs[c][:], pj[:])

            def xi_mms(pi, c, ms):
                for m in ms:
                    for g in range(2):
                        nc.tensor.matmul(
                            pi[:, g * 512:(g + 1) * 512],
                            wlt[:, m, c * 128:(c + 1) * 128],
                            xt[:, m, g * 512:(g + 1) * 512],
                            start=(m == 0), stop=(m == 3),
                        )

            def emit_xi(c):
                pi = psB.tile([128, 1024], F32, name="psi", tag="B")
                xi_mms(pi, c, range(4))
                act_leaky(ots[c][:], pi[:], bias=bls[:, c:c + 1])

            def emit_finish(c):
                # add the neighbor half on the DVE (fast + predictable
                # there) and store on the sync ring (idle after the early
                # mask loads -- keeping triggers off the busy ACT queue).
                of = outp.tile([128, 1024], FP16, name="otf", bufs=2)
                for g in range(2):
                    gs = slice(g * 512, (g + 1) * 512)
                    nc.vector.tensor_tensor(of[:, gs], ots[c][:, gs],
                                            xjs[c][:, gs], op=OP.add)
                nc.sync.dma_start(out_d[c * 128:(c + 1) * 128, :], of[:])

            def emit_tail(c):
                # last cout chunk: g-outer matmuls so the g0 half drains,
                # adds, and stores while g1 is still on the PE.
                pi = psB.tile([128, 1024], F32, name="psi", tag="B")
                of = outp.tile([128, 1024], FP16, name="otf", bufs=2)
                for g in range(2):
                    gs = slice(g * 512, (g + 1) * 512)
                    for m in range(4):
                        nc.tensor.matmul(
                            pi[:, gs], wlt[:, m, c * 128:(c + 1) * 128],
                            xt[:, m, gs], start=(m == 0), stop=(m == 3),
                        )
                for g in range(2):
                    gs = slice(g * 512, (g + 1) * 512)
                    act_leaky(ots[c][:, gs], pi[:, gs], bias=bls[:, c:c + 1])
                    nc.vector.tensor_tensor(of[:, gs], ots[c][:, gs],
                                            xjs[c][:, gs], op=OP.add)
                    ring = nc.sync if g == 0 else nc.scalar
                    ring.dma_start(out_d[c * 128:(c + 1) * 128, gs],
                                   of[:, gs])

            # PE order interleaves the first xi chunks into the
            # keep/xwq-latency window so the tensor engine never idles.
            pi0 = psB.tile([128, 1024], F32, name="psi", tag="B")
            xi_mms(pi0, 0, [0, 1, 2])
            emit_xj(0)
            xi_mms(pi0, 0, [3])
            act_leaky(ots[0][:], pi0[:], bias=bls[:, 0:1])
            emit_xj(1)
            pi1 = psB.tile([128, 1024], F32, name="psi", tag="B")
            xi_mms(pi1, 1, [0, 1])
            emit_xj(2)
            emit_finish(0)
            xi_mms(pi1, 1, [2, 3])
            act_leaky(ots[1][:], pi1[:], bias=bls[:, 1:2])
            emit_xj(3)
            emit_finish(1)
            emit_xi(2)
            emit_finish(2)
            emit_tail(3)

    nc.compile()
    return nc


def _get_nc(nn: int):
    key = (nn, SIM_SAFE, WARMUP_MM)
    if key not in _nc_cache:
        _nc_cache[key] = _build_nc(nn)
    return _nc_cache[key]


def _numpy_fallback(X, A, W_nb, b_nb, W_line, b_line, nn):
    def leaky(x):
        return np.where(x >= 0, x, NEG * x)

    Ab = A > 0
    keep = Ab & (np.cumsum(Ab.astype(np.int64), axis=1) <= nn)
    cnt = keep.sum(axis=1, keepdims=True).astype(X.dtype)
    nb_sum = keep.astype(X.dtype) @ X
    nb_mean = nb_sum / np.maximum(cnt, 1.0)
    xj = leaky(nb_mean @ W_nb.T + b_nb)
    xi = leaky(X @ W_line.T + b_line)
    return (xi + np.where(cnt > 0, xj, 0.0)).astype(np.float32)


def _pack_m(arr, nm):
    """[nm*128, w] -> [128, nm*w]: chunk m lands at columns [m*w:(m+1)*w]."""
    w = arr.shape[1]
    return np.ascontiguousarray(
        arr.reshape(nm, 128, w).transpose(1, 0, 2).reshape(128, nm * w))


def build_in_maps(X, A, W_nb, b_nb, W_line, b_line, nn):
    """Shard the full inputs into one input map per core."""
    import ml_dtypes
    f8 = ml_dtypes.float8_e4m3

    ones = np.ones((128, 128), np.float32)
    smq = np.concatenate(
        [ones, np.triu(ones), np.zeros((128, 128), np.float32)],
        axis=1).astype(f8)                                      # [128, 384]
    # Xw precomputed on host: psJ = sum over nn kept cands of BETA*Xw
    # must equal nb_mean @ W_nb.T + b_nb  =>  scale by 1/(BETA*nn).
    sx = np.float32(1.0 / (BETA * nn))
    Xw = (X[:C].astype(np.float32) @ W_nb.T.astype(np.float32)
          + b_nb.astype(np.float32)) * sx                       # [256, 512]
    xwq = _pack_m(Xw, 2).astype(f8)                             # [128, 1024]
    wlt = _pack_m(np.ascontiguousarray(W_line.T).astype(np.float16), 4)
    bls = np.ascontiguousarray(
        b_line.astype(np.float32).reshape(4, 128).T)            # [128, 4]

    Ab8 = ((A[:, :C] > 0).astype(np.float32)
           * np.float32(BETA)).astype(f8)                       # [N, 256]
    XT = np.ascontiguousarray(X.T.astype(np.float16))           # [512, N]
    in_maps = []
    for cix in range(NCORES):
        rows = slice(cix * R, (cix + 1) * R)
        blk = Ab8[rows]                                         # [1024, 256]
        at = np.ascontiguousarray(
            blk.reshape(2, 512, 2, 128)                         # [g, r', t, p]
               .transpose(3, 0, 2, 1).reshape(128, 2048))       # [p,(g,t,r')]
        xt = _pack_m(np.ascontiguousarray(XT[:, rows]), 4)      # [128, 4096]
        in_maps.append({
            "at": at, "smq": smq, "xwq": xwq, "bls": bls,
            "wlt": wlt, "xt": xt,
        })
    return in_maps


def _unshard_out(outs):
    """outs: per-core [512, 1024] fp16 outT -> full [N, 512] f32."""
    full = np.stack([np.asarray(o) for o in outs], axis=0)      # [8, 512, 1024]
    return np.ascontiguousarray(
        full.transpose(0, 2, 1).reshape(N, COUT)).astype(np.float32)


def kernel(**inputs) -> np.ndarray:
    global LAST_RESULT
    X = np.ascontiguousarray(np.asarray(inputs["X"], dtype=np.float32))
    A = np.ascontiguousarray(np.asarray(inputs["A"], dtype=np.int32))
    W_nb = np.asarray(inputs["W_nb"], dtype=np.float32)
    b_nb = np.asarray(inputs["b_nb"], dtype=np.float32)
    W_line = np.asarray(inputs["W_line"], dtype=np.float32)
    b_line = np.asarray(inputs["b_line"], dtype=np.float32)
    nn = int(np.asarray(inputs["neibor_num"]))

    # Fast path requires: every row reaches nn set bits within the first C
    # columns (=> keep-mask confined to [:, :C] and cnt == nn > 0 per row).
    fast = (
        X.shape == (N, CIN) and A.shape == (N, N) and 1 <= nn <= C
        and int(np.count_nonzero(A[:, :C] > 0, axis=1).min()) >= nn
    )
    if not fast:
        return _numpy_fallback(X, A, W_nb, b_nb, W_line, b_line, nn)

    import os

    in_maps = build_in_maps(X, A, W_nb, b_nb, W_line, b_line, nn)
    nc = _get_nc(nn)
    if os.environ.get("BASS_TRACE"):
        from concourse.bass_utils import run_bass_kernel_spmd
        res = run_bass_kernel_spmd(nc, in_maps, core_ids=list(range(NCORES)))
        LAST_RESULT = res
        return _unshard_out([r["out"] for r in res.results])
    outs = _run_cached(nc, nn, in_maps)
    return _unshard_out(outs)


_runner_cache = {}


def _run_cached(nc, nn, in_maps):
    """Execute the compiled program on the 8 cores, caching the jitted
    executable across calls (mirrors bass2jax.run_bass_via_pjrt's
    multi-core path; falls back to it on any setup error)."""
    import jax
    import concourse.mybir as mybir
    from concourse import bass2jax

    if nn not in _runner_cache:
        try:
            bass2jax.install_neuronx_cc_hook()
            part_name = (nc.partition_id_tensor.name
                         if nc.partition_id_tensor else None)
            in_names, out_names, out_avals, zero_shapes = [], [], [], []
            for alloc in nc.m.functions[0].allocations:
                if not isinstance(alloc, mybir.MemoryLocationSet):
                    continue
                name = alloc.memorylocations[0].name
                if alloc.kind == "ExternalInput":
                    if name != part_name:
                        in_names.append(name)
                elif alloc.kind == "ExternalOutput":
                    out_names.append(name)
                    np_dt = mybir.dt.np(alloc.dtype)
                    out_avals.append(jax.core.ShapedArray(
                        tuple(alloc.tensor_shape), np_dt))
                    zero_shapes.append((tuple(alloc.tensor_shape), np_dt))
            n_params = len(in_names)
            all_names = tuple(in_names + out_names
                              + ([part_name] if part_name else []))

            def _body(*args):
                operands = list(args)
                if part_name:
                    operands.append(bass2jax.partition_id_tensor())
                outs = bass2jax._bass_exec_p.bind(
                    *operands,
                    out_avals=tuple(out_avals),
                    in_names=all_names,
                    out_names=tuple(out_names),
                    lowering_input_output_aliases=(),
                    sim_require_finite=True,
                    sim_require_nnan=True,
                    nc=nc,
                )
                return tuple(outs)

            from jax.sharding import Mesh, PartitionSpec
            try:
                from jax.experimental.shard_map import shard_map
            except ImportError:
                from jax.shard_map import shard_map
            devices = jax.devices()[:NCORES]
            assert len(devices) == NCORES
            mesh = Mesh(np.asarray(devices), ("core",))
            n_outs = len(out_names)
            sharded = jax.jit(
                shard_map(_body, mesh=mesh,
                          in_specs=(PartitionSpec("core"),) * (n_params + n_outs),
                          out_specs=(PartitionSpec("core"),) * n_outs,
                          check_rep=False),
                donate_argnums=tuple(range(n_params, n_params + n_outs)),
                keep_unused=True,
            )
            _runner_cache[nn] = (sharded, in_names, out_names, zero_shapes)
        except Exception:
            _runner_cache[nn] = None
    cached = _runner_cache[nn]
    if cached is None:
        from concourse.bass_utils import run_bass_kernel_spmd
        res = run_bass_kernel_spmd(nc, in_maps, core_ids=list(range(NCORES)))
        return [r["out"] for r in res.results]
    sharded, in_names, out_names, zero_shapes = cached
    concat_in = [np.concatenate([np.asarray(m[name]) for m in in_maps], axis=0)
                 for name in in_names]
    concat_zeros = [np.zeros((NCORES * sh[0],) + sh[1:], dt)
                    for sh, dt in zero_shapes]
    out_arrs = sharded(*concat_in, *concat_zeros)
    oi = out_names.index("out")
    full = np.asarray(out_arrs[oi]).reshape(NCORES, 512, R)
    return [full[c] for c in range(NCORES)]


if __name__ == "__main__":
    rng = np.random.default_rng(0)
    X = rng.standard_normal((N, CIN), dtype=np.float32)
    A = (rng.random((N, N)) < 0.5).astype(np.int32)
    W_nb = rng.standard_normal((COUT, CIN), dtype=np.float32) * 0.04
    b_nb = rng.standard_normal(COUT, dtype=np.float32) * 0.04
    W_line = rng.standard_normal((COUT, CIN), dtype=np.float32) * 0.04
    b_line = rng.standard_normal(COUT, dtype=np.float32) * 0.04
    out = kernel(X=X, A=A, W_nb=W_nb, b_nb=b_nb, W_line=W_line,
                 b_line=b_line, neibor_num=64)
    exp = _numpy_fallback(X, A, W_nb, b_nb, W_line, b_line, 64)
    err = np.abs(out - exp).max() / np.abs(exp).max()
    print("self-test rel err:", err)
